# revision 1
# baseline (speedup 1.0000x reference)
"""Trainium2 Bass kernel for CrossAttention (B=4, S=S_ctx=2048, D=1024, H=16, Hd=64).

Sharding: 8 cores = batch (4) x head-group (2 groups of 8 heads).
Each core computes, for its (b, g):
    q = x_b @ qw_g          (per-head mean-centering folded into qw on host)
    k = ctx_b @ kw_g        (same)
    v = ctx_b @ vw_g
    per-head LN (rstd only; mean is zero by construction), RoPE on q,
    softmax(q k^T / 8) v per head, partial out-proj with this group's proj_w
    rows.  Host sums the two group partials per batch and adds proj bias.

v2 structure (vs baseline): three phases tuned for engine streaming.
  A: K/V/Q projections, software-pipelined emission (PE transposes of
     tile t+1 are issued before tile t's matmuls so the PE never
     head-of-line blocks on the ScalarE PSUM->SBUF copies).  rstd-mul
     writes the bf16 LN'd tensors directly from PSUM (no extra copy).
     Out-transposes run in bf16 (1 cyc/row), in-transposes f32r (1.5).
  B: pure attention: logits -> exp -> PV only on PE/ScalarE; PSUM is
     exactly 2x2-bank logits bufs + 2x2-bank output accumulators.
     Softmax denominators leave PSUM via gpsimd DMA into a [8, SC]
     collect tile; ONE batched DVE reciprocal per sc-block (instead of
     32 x 3.4us single-partition reciprocals), broadcast back with
     gpsimd partition_broadcast (no PE broadcast matmuls).
  C: trailing out-projection over the full S (aoT kept in SBUF), a
     pure PE streak with no PSUM contention with phase B.
"""

import numpy as np
from contextlib import ExitStack

import concourse.bacc as bacc
import concourse.bass as bass
import concourse.tile as tile
from concourse import mybir
from concourse.bass_utils import run_bass_kernel_spmd
from concourse.masks import make_identity

F32 = mybir.dt.float32
F32R = mybir.dt.float32r
BF16 = mybir.dt.bfloat16
AF = mybir.ActivationFunctionType

B, S, DIM = 4, 2048, 1024
H, HD = 16, 64
G = 2                  # head groups (tensor-parallel dim)
HL = H // G            # heads per core = 8
DL = HL * HD           # local head dims = 512
P = 128
NT = S // P            # 16 token tiles
NR = DIM // P          # 8 channel tiles
EPS = 1e-5
NSC = 4
SC = S // NSC          # 512 queries per attention block

_program_cache = {}
LAST_RUN = None        # BassKernelResults of most recent run (for test harness)


def _mk_ap(ap, dims):
    """Raw AP on the same tensor/offset with explicit [step, count] dims."""
    return bass.AP(tensor=ap.tensor, offset=ap.offset, ap=list(dims))


def _build_program(ln_affine_q, ln_affine_k, trace=False):
    nc = bacc.Bacc(None, target_bir_lowering=False, debug=False)

    x_d = nc.dram_tensor("x", [S, DIM], F32R, kind="ExternalInput")
    ctx_d = nc.dram_tensor("ctx", [S, DIM], F32R, kind="ExternalInput")
    qw_d = nc.dram_tensor("qw", [DIM, DL], F32R, kind="ExternalInput")
    kw_d = nc.dram_tensor("kw", [DIM, DL], F32R, kind="ExternalInput")
    vw_d = nc.dram_tensor("vw", [DIM, DL], F32R, kind="ExternalInput")
    pw_d = nc.dram_tensor("pw", [DL, DIM], BF16, kind="ExternalInput")
    cos_d = nc.dram_tensor("cos", [S, HD // 2], F32, kind="ExternalInput")
    sin_d = nc.dram_tensor("sin", [S, HD // 2], F32, kind="ExternalInput")
    qs_d = nc.dram_tensor("qs", [HD], F32, kind="ExternalInput")
    qb_d = nc.dram_tensor("qb", [HD], F32, kind="ExternalInput")
    ks_d = nc.dram_tensor("ks", [HD], F32, kind="ExternalInput")
    kb_d = nc.dram_tensor("kb", [HD], F32, kind="ExternalInput")
    y_d = nc.dram_tensor("y", [S, DIM], F32, kind="ExternalOutput")
    deni_d = nc.dram_tensor("den_inv_scratch", [NSC, HL // 2, 2, SC], F32)

    with tile.TileContext(nc) as tc, ExitStack() as top:
        const = top.enter_context(tc.tile_pool(name="const", bufs=1))
        identF = const.tile([P, P], F32)
        make_identity(nc, identF[:])
        identR_t = const.tile([P, P], F32R)
        nc.scalar.copy(identR_t[:], identF[:])
        identR = identR_t[:]
        identB = const.tile([P, P], BF16)
        nc.vector.tensor_copy(identB[:], identF[:])

        eps_sb = const.tile([P, 1], F32)
        nc.vector.memset(eps_sb[:], EPS)

        cos_sb = const.tile([P, NT, HD // 2], F32)
        sin_sb = const.tile([P, NT, HD // 2], F32)
        nc.sync.dma_start(cos_sb[:], cos_d[:].rearrange("(i p) f -> p i f", p=P))
        nc.sync.dma_start(sin_sb[:], sin_d[:].rearrange("(i p) f -> p i f", p=P))

        ln_tiles = {}
        for flag, s_t, b_t, key in (
            (ln_affine_q, qs_d, qb_d, "q"),
            (ln_affine_k, ks_d, kb_d, "k"),
        ):
            if flag:
                st = const.tile([P, HD], F32)
                bt = const.tile([P, HD], F32)
                nc.gpsimd.dma_start(st[:], s_t[:].partition_broadcast(P))
                nc.gpsimd.dma_start(bt[:], b_t[:].partition_broadcast(P))
                ln_tiles[key] = (st, bt)

        persist = top.enter_context(tc.tile_pool(name="persist", bufs=1))
        QT = persist.tile([P, HL // 2, S], BF16)         # [d-par, pair, s]
        KT = persist.tile([P, HL // 2, S], BF16)         # [d-par, pair, t]
        Vaug = persist.tile([P, NT, HL, HD + 1], BF16)   # [t-par, t-tile, h, e|1]
        aoT = persist.tile([P, DL // P, S], BF16)        # attn out, d-major
        pw_sb = persist.tile([P, DL // P, DIM], BF16)
        nc.vector.memset(Vaug[:, :, :, HD : HD + 1], 1.0)
        nc.sync.dma_start(pw_sb[:], pw_d[:].rearrange("(r p) n -> p r n", p=P))

        def apply_affine(nat3, key):
            if key in ln_tiles:
                st, bt = ln_tiles[key]
                stb = _mk_ap(st[:], [st[:].ap[0], [0, HL], [1, HD]])
                btb = _mk_ap(bt[:], [bt[:].ap[0], [0, HL], [1, HD]])
                nc.vector.tensor_mul(nat3, nat3, stb)
                nc.vector.tensor_add(nat3, nat3, btb)

        # ================= phase A: projections =================
        with ExitStack() as ph:
            tp = ph.enter_context(tc.tile_pool(name="tin", bufs=3))
            xtp = ph.enter_context(tc.tile_pool(name="xt", bufs=3))
            wp = ph.enter_context(tc.tile_pool(name="w", bufs=1))
            work = ph.enter_context(tc.tile_pool(name="work", bufs=3))
            tr_ps = ph.enter_context(tc.tile_pool(name="tr_ps", bufs=3, space="PSUM"))
            mm_ps = ph.enter_context(tc.tile_pool(name="mm_ps", bufs=3, space="PSUM"))
            ot_ps = ph.enter_context(tc.tile_pool(name="ot_ps", bufs=2, space="PSUM"))

            kw_sb = wp.tile([P, NR, DL], F32R)
            vw_sb = wp.tile([P, NR, DL], F32R)
            qw_sb = wp.tile([P, NR, DL], F32R)
            nc.sync.dma_start(kw_sb[:], kw_d[:].rearrange("(r p) d -> p r d", p=P))
            nc.sync.dma_start(vw_sb[:], vw_d[:].rearrange("(r p) d -> p r d", p=P))
            nc.sync.dma_start(qw_sb[:], qw_d[:].rearrange("(r p) d -> p r d", p=P))

            def load_transpose(inp_dram, t):
                """DMA token-tile t, PE-transpose to channel-major [P, NR, P]."""
                t_in = tp.tile([P, DIM], F32R, tag="t_in")
                nc.sync.dma_start(t_in[:], inp_dram[t * P : (t + 1) * P, :])
                t_inR = t_in[:]
                xt = xtp.tile([P, NR, P], F32R, tag="xt")
                for r2 in range(2):
                    ps4 = tr_ps.tile([P, 4 * P], F32R, tag="ps4")
                    for j in range(4):
                        nc.tensor.transpose(
                            ps4[:, j * P : (j + 1) * P],
                            t_inR.rearrange("p (r c) -> p r c", c=P)[
                                :, r2 * 4 + j, :],
                            identR,
                        )
                    nc.scalar.copy(xt[:, r2 * 4 : (r2 + 1) * 4, :], ps4[:])
                return xt

            def rstd_of(ps_nat, dst):
                """dst = 1/sqrt(mean(nat^2 per head) + eps); ps_nat [P, DL] PSUM."""
                sq = work.tile([P, DL], F32, tag="sq")
                nc.scalar.square(sq[:], ps_nat[:])
                sums = work.tile([P, HL], F32, tag="sums")
                nc.vector.tensor_reduce(
                    sums[:],
                    sq[:].rearrange("p (h d) -> p h d", h=HL),
                    axis=mybir.AxisListType.X,
                    op=mybir.AluOpType.add,
                )
                sdt = work.tile([P, HL], F32, tag="sdt")
                nc.scalar.activation(
                    sdt[:], sums[:], AF.Sqrt, bias=eps_sb[:], scale=1.0 / HD
                )
                nc.vector.reciprocal(dst, sdt[:])

            def out_transpose(nat_b, OT, t):
                """PE-transpose token-major bf16 [P, DL] into OT[:, :, t*P...]."""
                for r4 in range(DL // P):
                    psT = ot_ps.tile([P, P], BF16, tag="psT")
                    nc.tensor.transpose(
                        psT[:], nat_b[:, r4 * P : (r4 + 1) * P], identB[:]
                    )
                    nc.scalar.copy(OT[:, r4, t * P : (t + 1) * P], psT[:])

            def kv_post(xt, t):
                ps_k = mm_ps.tile([P, DL], F32, tag="ps")
                for r in range(NR):
                    nc.tensor.matmul(
                        ps_k[:], xt[:, r, :], kw_sb[:, r, :],
                        start=(r == 0), stop=(r == NR - 1),
                    )
                ps_v = mm_ps.tile([P, DL], F32, tag="ps")
                for r in range(NR):
                    nc.tensor.matmul(
                        ps_v[:], xt[:, r, :], vw_sb[:, r, :],
                        start=(r == 0), stop=(r == NR - 1),
                    )
                nc.scalar.copy(
                    Vaug[:, t, :, 0:HD],
                    ps_v[:].rearrange("p (h d) -> p h d", h=HL),
                )
                rstd = work.tile([P, HL], F32, tag="rstd")
                rstd_of(ps_k, rstd[:])
                rb = _mk_ap(rstd[:], [rstd[:].ap[0], [1, HL], [0, HD]])
                k_nat = work.tile([P, DL], BF16, tag="k_nat")
                k3 = k_nat[:].rearrange("p (h d) -> p h d", h=HL)
                nc.vector.tensor_mul(
                    k3, ps_k[:].rearrange("p (h d) -> p h d", h=HL), rb
                )
                apply_affine(k3, "k")
                return k_nat, KT

            def q_post(xt, t):
                ps_q = mm_ps.tile([P, DL], F32, tag="ps")
                for r in range(NR):
                    nc.tensor.matmul(
                        ps_q[:], xt[:, r, :], qw_sb[:, r, :],
                        start=(r == 0), stop=(r == NR - 1),
                    )
                rstd = work.tile([P, HL], F32, tag="rstd")
                rstd_of(ps_q, rstd[:])
                rb = _mk_ap(rstd[:], [rstd[:].ap[0], [1, HL], [0, HD]])
                q_ln = work.tile([P, DL], F32, tag="q_ln")
                q3 = q_ln[:].rearrange("p (h d) -> p h d", h=HL)
                nc.vector.tensor_mul(
                    q3, ps_q[:].rearrange("p (h d) -> p h d", h=HL), rb
                )
                apply_affine(q3, "q")
                # RoPE: view [p, h, 2, 32]
                qcos = work.tile([P, DL], F32, tag="qcos")
                qsin = work.tile([P, DL], F32, tag="qsin")
                cb = _mk_ap(cos_sb[:, t, :],
                            [cos_sb[:].ap[0], [0, HL], [0, 2], [1, HD // 2]])
                sb = _mk_ap(sin_sb[:, t, :],
                            [sin_sb[:].ap[0], [0, HL], [0, 2], [1, HD // 2]])
                q4 = q_ln[:].rearrange("p (h two f) -> p h two f", h=HL, two=2)
                qcos4 = qcos[:].rearrange("p (h two f) -> p h two f", h=HL, two=2)
                qsin4 = qsin[:].rearrange("p (h two f) -> p h two f", h=HL, two=2)
                nc.vector.tensor_mul(qcos4, q4, cb)
                nc.vector.tensor_mul(qsin4, q4, sb)
                q_rope = work.tile([P, DL], BF16, tag="q_rope")
                qr4 = q_rope[:].rearrange("p (h two f) -> p h two f", h=HL, two=2)
                nc.vector.tensor_sub(
                    qr4[:, :, 0, :], qcos4[:, :, 0, :], qsin4[:, :, 1, :]
                )
                nc.vector.tensor_add(
                    qr4[:, :, 1, :], qsin4[:, :, 0, :], qcos4[:, :, 1, :]
                )
                return q_rope, QT

            # software-pipelined emission: PE order per step is
            # transposes(i+1), proj-matmuls(i), out-transposes(i-1) so the
            # PE never waits on the ScalarE xt-copy or the DVE rstd chain.
            sched = [(ctx_d, t, kv_post) for t in range(NT)] + \
                    [(x_d, t, q_post) for t in range(NT)]
            xt_cur = load_transpose(sched[0][0], sched[0][1])
            ot_prev = None
            for i, (src, t, post) in enumerate(sched):
                xt_next = None
                if i + 1 < len(sched):
                    xt_next = load_transpose(sched[i + 1][0], sched[i + 1][1])
                nat, OT = post(xt_cur, t)
                if ot_prev is not None:
                    out_transpose(ot_prev[0][:], ot_prev[1], ot_prev[2])
                ot_prev = (nat, OT, t)
                xt_cur = xt_next
            out_transpose(ot_prev[0][:], ot_prev[1], ot_prev[2])

        # ================= phase B: attention =================
        with ExitStack() as ph:
            l_ps = ph.enter_context(tc.tile_pool(name="l_ps", bufs=2, space="PSUM"))
            o_ps = ph.enter_context(tc.tile_pool(name="o_ps", bufs=2, space="PSUM"))
            ex_pool = ph.enter_context(tc.tile_pool(name="ex", bufs=3))
            num_pool = ph.enter_context(tc.tile_pool(name="num", bufs=6))
            d_pool = ph.enter_context(tc.tile_pool(name="den", bufs=8))
            nrm_pool = ph.enter_context(tc.tile_pool(name="nrm", bufs=4))

            for sc in range(NSC):
                dens = []
                nums = []
                for r in range(HL // 2):
                    ps_o = o_ps.tile([HD + 1, 2, SC], F32, tag="ps_o")
                    for t in range(NT):
                        ps_l = l_ps.tile([P, 2 * SC], F32, tag="ps_l")
                        nc.tensor.matmul(
                            ps_l[:, 0:SC],
                            KT[0:HD, r, t * P : (t + 1) * P],
                            QT[0:HD, r, sc * SC : (sc + 1) * SC],
                            start=True, stop=True,
                            tile_position=(0, 0),
                        )
                        nc.tensor.matmul(
                            ps_l[:, SC : 2 * SC],
                            KT[HD:P, r, t * P : (t + 1) * P],
                            QT[HD:P, r, sc * SC : (sc + 1) * SC],
                            start=True, stop=True,
                            tile_position=(HD, 0),
                        )
                        ex = ex_pool.tile([P, 2 * SC], BF16, tag="ex")
                        nc.scalar.activation(
                            ex[:], ps_l[:], AF.Exp, scale=1.0 / np.sqrt(HD)
                        )
                        for j in range(2):
                            nc.tensor.matmul(
                                ps_o[:, j, :],
                                Vaug[:, t, 2 * r + j, :],
                                ex[:, j * SC : (j + 1) * SC],
                                start=(t == 0), stop=(t == NT - 1),
                            )
                    # drain ps_o: numerator + den row to SBUF, recip off-path
                    num = num_pool.tile([HD, 2, SC], BF16, tag="num")
                    for j in range(2):
                        nc.vector.tensor_copy(num[:, j, :], ps_o[0:HD, j, :])
                    nums.append(num)
                    denS = d_pool.tile([HD + 1, 2, SC], F32, tag="denS")
                    nc.vector.tensor_copy(
                        denS[HD : HD + 1, :, :], ps_o[HD : HD + 1, :, :]
                    )
                    denSi = d_pool.tile([HD + 1, 2, SC], F32, tag="denSi")
                    nc.vector.reciprocal(
                        denSi[HD : HD + 1, :, :], denS[HD : HD + 1, :, :]
                    )
                    dens.append(denSi)
                    nc.gpsimd.dma_start(
                        deni_d[sc, r, :, :], denSi[HD : HD + 1, :, :]
                    )
                # sc end: broadcast inverse dens, normalize into aoT
                for r in range(HL // 2):
                    for j in range(2):
                        denB = nrm_pool.tile([HD, SC], F32, tag="denB")
                        nc.gpsimd.dma_start(
                            denB[:], deni_d[sc, r, j, :].partition_broadcast(HD)
                        )
                        if j == 0:
                            nc.vector.tensor_mul(
                                aoT[0:HD, r, sc * SC : (sc + 1) * SC],
                                nums[r][:, 0, :], denB[:],
                            )
                        else:
                            tmpB = nrm_pool.tile([HD, SC], BF16, tag="tmpB")
                            nc.vector.tensor_mul(tmpB[:], nums[r][:, 1, :], denB[:])
                            nc.gpsimd.dma_start(
                                aoT[HD:P, r, sc * SC : (sc + 1) * SC], tmpB[:]
                            )

        # ================= phase C: out-projection =================
        with ExitStack() as ph:
            y_ps = ph.enter_context(tc.tile_pool(name="y_ps", bufs=3, space="PSUM"))
            y_pool = ph.enter_context(tc.tile_pool(name="y", bufs=3))
            for si in range(NT):
                y_sb = y_pool.tile([P, DIM], F32, tag="y_sb")
                for n2 in range(2):
                    ps_y = y_ps.tile([P, DIM // 2], F32, tag="ps_y")
                    for rr in range(DL // P):
                        nc.tensor.matmul(
                            ps_y[:],
                            aoT[:, rr, si * P : (si + 1) * P],
                            pw_sb[:, rr, n2 * (DIM // 2) : (n2 + 1) * (DIM // 2)],
                            start=(rr == 0), stop=(rr == DL // P - 1),
                        )
                    nc.vector.tensor_copy(
                        y_sb[:, n2 * (DIM // 2) : (n2 + 1) * (DIM // 2)], ps_y[:]
                    )
                nc.sync.dma_start(y_d[si * P : (si + 1) * P, :], y_sb[:])

    nc.compile()
    return nc


def _center_mat():
    m = np.eye(HD, dtype=np.float64) - np.ones((HD, HD), dtype=np.float64) / HD
    return np.kron(np.eye(H, dtype=np.float64), m)  # [DIM, DIM] block-diag


def kernel(x, context, q_w, kv_w, qn_scale, qn_bias, kn_scale, kn_bias,
           proj_w, proj_b, _trace=False):
    global LAST_RUN
    x = np.asarray(x, np.float32)
    context = np.asarray(context, np.float32)
    q_w = np.asarray(q_w, np.float32)
    kv_w = np.asarray(kv_w, np.float32)
    proj_w = np.asarray(proj_w, np.float32)
    proj_b = np.asarray(proj_b, np.float32)
    qn_scale = np.asarray(qn_scale, np.float32)
    qn_bias = np.asarray(qn_bias, np.float32)
    kn_scale = np.asarray(kn_scale, np.float32)
    kn_bias = np.asarray(kn_bias, np.float32)

    ln_affine_q = not (np.all(qn_scale == 1.0) and np.all(qn_bias == 0.0))
    ln_affine_k = not (np.all(kn_scale == 1.0) and np.all(kn_bias == 0.0))

    key = (ln_affine_q, ln_affine_k)
    if key not in _program_cache:
        _program_cache[key] = _build_program(*key)
    nc = _program_cache[key]

    C = _center_mat()
    qw_c = (q_w.astype(np.float64) @ C).astype(np.float32)
    kw_c = (kv_w[:, :DIM].astype(np.float64) @ C).astype(np.float32)
    vw_full = np.ascontiguousarray(kv_w[:, DIM:])

    inv_freq = 1.0 / (10000.0 ** (np.arange(0, HD, 2, dtype=np.float32) / HD))
    ang = np.arange(S, dtype=np.float32)[:, None] * inv_freq
    cos_t = np.cos(ang).astype(np.float32)
    sin_t = np.sin(ang).astype(np.float32)

    in_maps = []
    for core in range(B * G):
        b, g = divmod(core, G)
        sl = slice(g * DL, (g + 1) * DL)
        in_maps.append({
            "x": np.ascontiguousarray(x[b]),
            "ctx": np.ascontiguousarray(context[b]),
            "qw": np.ascontiguousarray(qw_c[:, sl]),
            "kw": np.ascontiguousarray(kw_c[:, sl]),
            "vw": np.ascontiguousarray(vw_full[:, sl]),
            "pw": np.ascontiguousarray(proj_w[sl, :]).astype(mybir.dt.np(BF16)),
            "cos": cos_t, "sin": sin_t,
            "qs": qn_scale, "qb": qn_bias, "ks": kn_scale, "kb": kn_bias,
        })

    LAST_RUN = run_bass_kernel_spmd(
        nc, in_maps, list(range(B * G)), trace=_trace
    )
    res = LAST_RUN.results

    out = np.zeros((B, S, DIM), np.float32)
    for core in range(B * G):
        out[core // G] += res[core]["y"]
    out += proj_b[None, None, :]
    return out

